# revision 4
# baseline (speedup 1.0000x reference)
"""Trainium2 Bass kernel v2: 15x15 valid cross-correlation, 4096x4096 f32 in,
[4082, 4082] out, via 16-way 32x32 TensorE array packing (tile_position).

Strategy
--------
Column-shard output across 8 cores (512 cols each, input slab carries its own
14-col halo). Per core, the PE array is split into 16 independent 32x32 tiles
(4 row-groups x 4 col-groups). Tile (rg, cg) convolves its own 32-row input
slab (s = 4*rg + cg) against a [32, 18] banded stationary matrix per kernel
column dj:

    B_dj[k, i] = w[k - i, dj]  (0 <= k - i < 15),  i < 18 output rows/slab

15 dj-rounds accumulate in PSUM (moving operand = dj-shifted [32, 512] slice
of the slab). One pass = 16 slabs x 18 rows = 288 output rows, 15 passes
cover 4082 (padded to 4320). Useful MACs/cycle = 16*15*18 = 4320 vs 1710 for
the single 128-wide band -> ~2.5x fewer PE cycles.

PSUM: tile (rg, cg) -> bank rg, partitions [32cg, 32cg+18). One start=True
per bank per pass (first MM clears whole-bank has_written; every other
tile's first write lands as overwrite-where-unset). dj=0 uses a zero-padded
M=32 stationary so all 128 partitions get defined values for eviction.

Inputs stream as bf16 (halves HBM traffic; rel err ~3e-3 << 2e-2 tol).
"""

import sys

import numpy as np

sys.path.insert(0, "/opt/trn_rl_repo")

H = W = 4096
KH = KW = 15
OH = OW = H - KH + 1  # 4082
NCORES = 8
COLS_PER_CORE = 512
IN_COLS = 528  # 512 + 14 halo + 2 pad
SLAB_H = 32  # input rows per tile window
M_OUT = 18  # output rows per tile (32 - 14)
NTILES = 16
ROWS_PER_PASS = NTILES * M_OUT  # 288
NPASS = (OH + ROWS_PER_PASS - 1) // ROWS_PER_PASS  # 15
# Last pass only needs rows 4032..4081 -> a single row-group (4 slabs, 72
# rows) instead of 16 slabs.
NRG_LAST = 1
NBANKS_TOT = (NPASS - 1) * 4 + NRG_LAST  # 57
Y_ROWS = NBANKS_TOT * 128  # padded: one 128-row block per evicted bank
XS_ROWS = (NPASS - 1) * ROWS_PER_PASS + (4 * NRG_LAST - 1) * M_OUT + SLAB_H  # 4118
PAD_COLS = (NCORES - 1) * COLS_PER_CORE + IN_COLS  # 4112


def _build_bass(n_reps=1):
    import concourse.mybir as mybir
    from concourse import bacc
    from concourse.tile import TileContext
    from concourse.tile_rust import add_dep_helper

    from concourse.ap import AP as APcls

    f32 = mybir.dt.float32
    bf16 = mybir.dt.bfloat16

    nc = bacc.Bacc()
    xs = nc.declare_dram_parameter("xs", [XS_ROWS, IN_COLS], bf16, isOutput=False)
    xs_h = xs[:, :].tensor
    Tm = nc.declare_dram_parameter("Tm", [128, KW * 32], bf16, isOutput=False)
    y = nc.declare_dram_parameter("y", [Y_ROWS, COLS_PER_CORE], bf16, isOutput=True)

    with TileContext(nc) as tc:
        with (
            tc.tile_pool(name="const", bufs=1) as cpool,
            tc.tile_pool(name="xwin", bufs=3) as xpool,
            tc.tile_pool(name="obuf", bufs=8) as opool,
            tc.tile_pool(name="psum", bufs=2, space="PSUM") as ppool,
        ):
            T_sb = cpool.tile([128, KW * 32], bf16)
            nc.sync.dma_start(T_sb[:], Tm[:, :])

            prev_mm = None
            for _rep in range(n_reps):
                for p in range(NPASS):
                    r0 = ROWS_PER_PASS * p
                    nrg = 4 if p < NPASS - 1 else NRG_LAST
                    xw = xpool.tile([128, 4 * IN_COLS], bf16)
                    for rg in range(nrg):
                        # One DMA per row-group: 4 slab blocks (18-row step,
                        # 32-row overlapping windows) land in the 4 column
                        # blocks of partitions [32rg, 32rg+32).
                        dst = xw[32 * rg : 32 * rg + 32, :].rearrange(
                            "p (b c) -> p b c", b=4
                        )
                        srcap = APcls(
                            xs_h,
                            (r0 + 72 * rg) * IN_COLS,
                            [[IN_COLS, SLAB_H],
                             [M_OUT * IN_COLS, 4],
                             [1, IN_COLS]],
                        )
                        nc.sync.dma_start(dst, srcap)
                    ps = [
                        ppool.tile([128, COLS_PER_CORE], f32, name=f"ps{rg}")
                        for rg in range(4)
                    ]
                    for dj in range(KW):
                        m = 32 if dj == 0 else M_OUT
                        for rg in range(nrg):
                            for cg in range(4):
                                mm = nc.tensor.matmul(
                                    ps[rg][32 * cg : 32 * cg + m, :],
                                    lhsT=T_sb[32 * rg : 32 * rg + 32,
                                              32 * dj : 32 * dj + m],
                                    rhs=xw[32 * rg : 32 * rg + 32,
                                           IN_COLS * cg + dj :
                                           IN_COLS * cg + dj + COLS_PER_CORE],
                                    start=(dj == 0),
                                    stop=(dj == KW - 1),
                                    tile_position=(32 * rg, 32 * cg),
                                )
                                # Pin round-major PE order: same-subarray
                                # repeats must stay 16 apart or the in-order
                                # queue stalls a full stream per repeat.
                                if prev_mm is not None:
                                    add_dep_helper(
                                        mm.ins, prev_mm, sync=False,
                                        reason="round-major PE order",
                                    )
                                prev_mm = mm.ins
                    for rg in range(nrg):
                        ob = opool.tile([128, COLS_PER_CORE], bf16)
                        # Bias is added on the host. All-DVE eviction: 4 x
                        # ~0.73us/pass fits under the 3.2us matmul stream
                        # (ACT copies from PSUM measured ~1.3us/pass slower).
                        nc.vector.tensor_copy(ob[:, :], ps[rg][:, :])
                        # One full-bank DMA (junk rows 18..31 of each block
                        # included; host strips them).
                        yr = (4 * p + rg) * 128  # p==NPASS-1 has rg==0 only
                        nc.sync.dma_start(y[yr : yr + 128, :], ob[:, :])

    nc.finalize()
    return nc


def _host_prep(x, w, b):
    import ml_dtypes

    x = np.asarray(x, dtype=np.float32)
    w = np.asarray(w, dtype=np.float32)
    b = np.asarray(b, dtype=np.float32)

    x_pad = np.zeros((XS_ROWS, PAD_COLS), np.float32)
    x_pad[:H, :W] = x
    x_bf = x_pad.astype(ml_dtypes.bfloat16)

    T_np = np.zeros((128, KW * 32), np.float32)
    i = np.arange(M_OUT)
    for g in range(4):
        for dj in range(KW):
            for di in range(KH):
                T_np[32 * g + i + di, 32 * dj + i] = w[di, dj]
    T_bf = T_np.astype(ml_dtypes.bfloat16)

    in_maps = []
    for c in range(NCORES):
        slab = np.ascontiguousarray(
            x_bf[:, COLS_PER_CORE * c : COLS_PER_CORE * c + IN_COLS]
        )
        in_maps.append({"xs": slab, "Tm": T_bf})
    return in_maps


def _enable_ldw_opt():
    from concourse import bass_utils

    if getattr(bass_utils, "_ldw_opt_patched", False):
        return
    orig = bass_utils.run_command

    def patched(cmd, *a, **kw):
        if isinstance(cmd, list):
            cmd = [
                "--enable-ldw-opt=true" if c == "--enable-ldw-opt=false" else c
                for c in cmd
            ]
        return orig(cmd, *a, **kw)

    bass_utils.run_command = patched
    bass_utils._ldw_opt_patched = True


def run(x, w, b, n_reps=1):
    from concourse.bass_utils import run_bass_kernel_spmd

    nc = _build_bass(n_reps=n_reps)
    in_maps = _host_prep(x, w, b)
    res = run_bass_kernel_spmd(nc, in_maps, list(range(NCORES)))
    outs = []
    for c in range(NCORES):
        yv = np.asarray(res.results[c]["y"]).astype(np.float32)
        # one 128-row block per bank: (cg, 32, 512) with rows 18..31 junk;
        # slab s = 4*rg + cg at output rows 288p + 18s
        yv = yv.reshape(NBANKS_TOT, 4, 32, COLS_PER_CORE)[:, :, :M_OUT, :]
        yv = yv.reshape(NBANKS_TOT * 4 * M_OUT, COLS_PER_CORE)
        outs.append(yv)
    full = np.concatenate(outs, axis=1)[:OH, :OW]
    full += np.float32(np.asarray(b, np.float32)[0])
    return full


def kernel(x, w, b):
    return run(x, w, b)


# revision 5
# speedup vs baseline: 1.4929x; 1.4929x over previous
"""Trainium2 Bass kernel v2: 15x15 valid cross-correlation, 4096x4096 f32 in,
[4082, 4082] out, via 16-way 32x32 TensorE array packing (tile_position).

Strategy
--------
Column-shard output across 8 cores (512 cols each, input slab carries its own
14-col halo). Per core, the PE array is split into 16 independent 32x32 tiles
(4 row-groups x 4 col-groups). Tile (rg, cg) convolves its own 32-row input
slab (s = 4*rg + cg) against a [32, 18] banded stationary matrix per kernel
column dj:

    B_dj[k, i] = w[k - i, dj]  (0 <= k - i < 15),  i < 18 output rows/slab

15 dj-rounds accumulate in PSUM (moving operand = dj-shifted [32, 512] slice
of the slab). One pass = 16 slabs x 18 rows = 288 output rows; 14 full
passes + a 1-row-group pass cover 4082. Useful MACs/cycle = 16*15*18 = 4320
vs 1710 for the single 128-wide band -> ~2.5x fewer PE cycles; matmul
emission is pinned round-major (add_dep_helper) so the 16 subarrays stream
concurrently at the N=512 roofline (~211 ns/round measured).

PSUM: tile (rg, cg) -> bank rg, partitions [32cg, 32cg+18). One start=True
per bank per pass (first MM clears whole-bank has_written; every other
tile's first write lands as overwrite-where-unset). dj=0 uses a zero-padded
M=32 stationary so all 128 partitions get defined values for eviction.

Inputs stream as bf16 (halves HBM traffic; rel err ~3e-3 << 2e-2 tol).
"""

import sys

import numpy as np

sys.path.insert(0, "/opt/trn_rl_repo")

H = W = 4096
KH = KW = 15
OH = OW = H - KH + 1  # 4082
NCORES = 8
COLS_PER_CORE = 512
IN_COLS = 528  # 512 + 14 halo + 2 pad
SLAB_H = 32  # input rows per tile window
M_OUT = 18  # output rows per tile (32 - 14)
NTILES = 16
ROWS_PER_PASS = NTILES * M_OUT  # 288
NPASS = (OH + ROWS_PER_PASS - 1) // ROWS_PER_PASS  # 15
# Last pass only needs rows 4032..4081 -> a single row-group (4 slabs, 72
# rows) instead of 16 slabs.
NRG_LAST = 1
NBANKS_TOT = (NPASS - 1) * 4 + NRG_LAST  # 57
Y_ROWS = NBANKS_TOT * 128  # padded: one 128-row block per evicted bank
XS_ROWS = (NPASS - 1) * ROWS_PER_PASS + (4 * NRG_LAST - 1) * M_OUT + SLAB_H  # 4118
PAD_COLS = (NCORES - 1) * COLS_PER_CORE + IN_COLS  # 4112


def _build_bass(n_reps=1):
    import concourse.mybir as mybir
    from concourse import bacc
    from concourse.tile import TileContext
    from concourse.tile_rust import add_dep_helper

    from concourse.ap import AP as APcls

    f32 = mybir.dt.float32
    bf16 = mybir.dt.bfloat16

    nc = bacc.Bacc()
    xs = nc.declare_dram_parameter("xs", [XS_ROWS, IN_COLS], bf16, isOutput=False)
    xs_h = xs[:, :].tensor
    Tm = nc.declare_dram_parameter("Tm", [128, KW * 32], bf16, isOutput=False)
    y = nc.declare_dram_parameter("y", [Y_ROWS, COLS_PER_CORE], bf16, isOutput=True)

    with TileContext(nc) as tc:
        with (
            tc.tile_pool(name="const", bufs=1) as cpool,
            tc.tile_pool(name="xwin", bufs=3) as xpool,
            tc.tile_pool(name="obuf", bufs=8) as opool,
            tc.tile_pool(name="psum", bufs=2, space="PSUM") as ppool,
        ):
            T_sb = cpool.tile([128, KW * 32], bf16)
            nc.sync.dma_start(T_sb[:], Tm[:, :])

            prev_mm = None
            for _rep in range(n_reps):
                for p in range(NPASS):
                    r0 = ROWS_PER_PASS * p
                    nrg = 4 if p < NPASS - 1 else NRG_LAST
                    xw = xpool.tile([128, 4 * IN_COLS], bf16)
                    for rg in range(nrg):
                        # One DMA per row-group: 4 slab blocks (18-row step,
                        # 32-row overlapping windows) land in the 4 column
                        # blocks of partitions [32rg, 32rg+32).
                        dst = xw[32 * rg : 32 * rg + 32, :].rearrange(
                            "p (b c) -> p b c", b=4
                        )
                        srcap = APcls(
                            xs_h,
                            (r0 + 72 * rg) * IN_COLS,
                            [[IN_COLS, SLAB_H],
                             [M_OUT * IN_COLS, 4],
                             [1, IN_COLS]],
                        )
                        nc.sync.dma_start(dst, srcap)
                    ps = [
                        ppool.tile([128, COLS_PER_CORE], f32, name=f"ps{rg}")
                        for rg in range(4)
                    ]
                    for dj in range(KW):
                        m = 32 if dj == 0 else M_OUT
                        for rg in range(nrg):
                            for cg in range(4):
                                mm = nc.tensor.matmul(
                                    ps[rg][32 * cg : 32 * cg + m, :],
                                    lhsT=T_sb[32 * rg : 32 * rg + 32,
                                              32 * dj : 32 * dj + m],
                                    rhs=xw[32 * rg : 32 * rg + 32,
                                           IN_COLS * cg + dj :
                                           IN_COLS * cg + dj + COLS_PER_CORE],
                                    start=(dj == 0),
                                    stop=(dj == KW - 1),
                                    tile_position=(32 * rg, 32 * cg),
                                )
                                # Pin round-major PE order: same-subarray
                                # repeats must stay 16 apart or the in-order
                                # queue stalls a full stream per repeat.
                                if prev_mm is not None:
                                    add_dep_helper(
                                        mm.ins, prev_mm, sync=False,
                                        reason="round-major PE order",
                                    )
                                prev_mm = mm.ins
                    for rg in range(nrg):
                        ob = opool.tile([128, COLS_PER_CORE], bf16)
                        # Bias is added on the host. All-DVE eviction: 4 x
                        # ~0.73us/pass fits under the 3.2us matmul stream
                        # (ACT copies from PSUM measured ~1.3us/pass slower).
                        nc.vector.tensor_copy(ob[:, :], ps[rg][:, :])
                        # One full-bank DMA (junk rows 18..31 of each block
                        # included; host strips them).
                        yr = (4 * p + rg) * 128  # p==NPASS-1 has rg==0 only
                        nc.sync.dma_start(y[yr : yr + 128, :], ob[:, :])

    nc.finalize()
    return nc


def _host_prep(x, w, b):
    import ml_dtypes

    x = np.asarray(x, dtype=np.float32)
    w = np.asarray(w, dtype=np.float32)
    b = np.asarray(b, dtype=np.float32)

    x_pad = np.zeros((XS_ROWS, PAD_COLS), np.float32)
    x_pad[:H, :W] = x
    x_bf = x_pad.astype(ml_dtypes.bfloat16)

    T_np = np.zeros((128, KW * 32), np.float32)
    i = np.arange(M_OUT)
    for g in range(4):
        for dj in range(KW):
            for di in range(KH):
                T_np[32 * g + i + di, 32 * dj + i] = w[di, dj]
    T_bf = T_np.astype(ml_dtypes.bfloat16)

    in_maps = []
    for c in range(NCORES):
        slab = np.ascontiguousarray(
            x_bf[:, COLS_PER_CORE * c : COLS_PER_CORE * c + IN_COLS]
        )
        in_maps.append({"xs": slab, "Tm": T_bf})
    return in_maps


def _enable_ldw_opt():
    from concourse import bass_utils

    if getattr(bass_utils, "_ldw_opt_patched", False):
        return
    orig = bass_utils.run_command

    def patched(cmd, *a, **kw):
        if isinstance(cmd, list):
            cmd = [
                "--enable-ldw-opt=true" if c == "--enable-ldw-opt=false" else c
                for c in cmd
            ]
        return orig(cmd, *a, **kw)

    bass_utils.run_command = patched
    bass_utils._ldw_opt_patched = True


def run(x, w, b, n_reps=1):
    from concourse.bass_utils import run_bass_kernel_spmd

    nc = _build_bass(n_reps=n_reps)
    in_maps = _host_prep(x, w, b)
    res = run_bass_kernel_spmd(nc, in_maps, list(range(NCORES)))
    outs = []
    for c in range(NCORES):
        yv = np.asarray(res.results[c]["y"]).astype(np.float32)
        # one 128-row block per bank: (cg, 32, 512) with rows 18..31 junk;
        # slab s = 4*rg + cg at output rows 288p + 18s
        yv = yv.reshape(NBANKS_TOT, 4, 32, COLS_PER_CORE)[:, :, :M_OUT, :]
        yv = yv.reshape(NBANKS_TOT * 4 * M_OUT, COLS_PER_CORE)
        outs.append(yv)
    full = np.concatenate(outs, axis=1)[:OH, :OW]
    full += np.float32(np.asarray(b, np.float32)[0])
    return full


def kernel(x, w, b):
    return run(x, w, b)
